# revision 7
# baseline (speedup 1.0000x reference)
"""MinibatchDiscrimination TRN2 kernel (v3).

x: [512, 1024] f32, T: [1024, 1024] f32.
M = (x @ T).reshape(512, 64, 16); l1[i,j,k] = sum_d |M[i,kd]-M[j,kd]|
out[i,k] = sum_j exp(-l1[i,j,k]) - 1.

Sharding: batch rows split across 8 cores (64 each), no collectives; each
core's x^T copy is rolled so its own rows sit at local columns 0..63.

Pair-shared windows: local rows r0=2m, r1=2m+1 share j-window
[r0+2, r0+258). Row r0 covers pair distances [2,257], r1 covers [1,256];
the last 2 window columns are computed from both ends so they accumulate
row-side only (col-accumulate drops them); within-pair {2m,2m+1} pairs are
a tiny separate d1 pass added to both rows host-side. Coverage is exact —
no diagonal, no -1 correction.

|z| via relu: slot (row r, kd-tile t) computes relu(sigma*z) with
sigma=+1 on DVE (tensor_scalar subtract/max at the 4x fp16 perf mode) or
sigma=-1 on ACT (Relu(bias - in)). l1 = 2*sum relu - sigma*(G_j - G_i),
G[k,j] = sum_d M[j,kd]. The j-term rides one identity matmul per pair
(rhs = Gn2 = -sigma*G with the pair's two rows stacked on 128 partitions);
the i-term rides the exp's per-partition bias.

l1 PSUM [128, 256]: partitions 0-63 = row r0's 64 k's, 64-127 = r1's.
Four [128,32] one-hot weight-2 patterns (S32) target 32-aligned output
partition groups via explicit tile_position, ordered to reuse each loaded
pattern 4x. One Exp per pair (scale=-1, bias=G-term) emits E fp16 plus row
sums via accum_out; column sums accumulate on the PE into a persistent
PSUM tile (lhsT=Icol), initialized by a zero-weights matmul.

Everything on-chip is fp16 (hosts casts inputs); PSUM is fp32.
"""

import numpy as np

import concourse.bass as bass
import concourse.tile as tile
from concourse import mybir
from concourse import bass_utils

B = 512
F = 1024
KD = 1024  # NUM_KERNELS(64) * KERNEL_DIM(16)
NK = 64
N_CORES = 8
NI = B // N_CORES  # local rows per core (64)
NP = NI // 2  # row pairs per core (32)
NT = KD // 128  # kd tiles (8)
NF = F // 128  # f chunks (8)
W = 256  # shared j-window width per pair
JL = NI + W  # used local-j extent (320)
# (row, tile) diff slots computed on ScalarE (sigma=-1); rest on DVE (+1)
ACT_SLOTS = {(0, 0), (1, 0)}
EXP_LAG = 1  # pairs between l1 matmuls and their exp
COL_LAG = 2  # pairs between exp and its col-accumulate matmul

_FP32 = mybir.dt.float32
_FP16 = mybir.dt.float16


def _sigma(r, t):
    return -1.0 if (r, t) in ACT_SLOTS else 1.0


def _split_all_waits(nc):
    """walrus in this env encodes at most 1 sync wait per instruction: hoist
    extra waits onto same-engine NOPs inserted just before the instruction.
    Safe because waits are AND-ed stall conditions on the engine's sequencer
    and semaphores are monotonic."""
    count = 0
    for fn in nc.m.functions:
        for bb in fn.blocks:
            insts = list(bb.instructions)
            new = []
            changed = False
            for inst in insts:
                si = getattr(inst, "sync_info", None)
                waits = list(si.on_wait) if (si is not None and si.on_wait) else []
                if len(waits) > 1:
                    for w in waits[:-1]:
                        nop = mybir.InstNoOp(name=f"NOPW-{count}", ins=[], outs=[])
                        count += 1
                        nop.engine = inst.engine
                        nop.sync_info = mybir.SyncInfo(on_wait=[w], on_update=[])
                        nc.register_instruction(nop, overwrite=True)
                        new.append(nop)
                    si.on_wait = [waits[-1]]
                    changed = True
                new.append(inst)
            if changed:
                bb.instructions[:] = new


def _patch_drain_wait_limit():
    if getattr(tile.TileContext, "_wait_split_patched", False):
        return
    orig = tile.TileContext.schedule_and_allocate

    def schedule_and_allocate(self, *a, **k):
        r = orig(self, *a, **k)
        _split_all_waits(self.nc)
        return r

    tile.TileContext.schedule_and_allocate = schedule_and_allocate
    tile.TileContext._wait_split_patched = True


def build_s_matrices():
    """S32 [128,128]: four one-hot weight-2 d-sum patterns; pattern b
    (cols 32b..32b+32) maps partition p -> col 8b + p//16, serving kd-tile
    t with b = t%4 at output partition group 32*(t//4) + 64*row.
    Sg [128, 16*64]: slice s=8r+t maps p -> col 8t + p//16 with value
    -sigma(r,t); matmuls vs mt[t] build Gn2[64r+k, j] = -sigma*G[k,j].
    I128: identity (per-pair Gn_j injection). Icol[p, p%64] = 1 (col sums).
    """
    S32 = np.zeros((128, 128), dtype=np.float16)
    for bq in range(4):
        for p in range(128):
            S32[p, 32 * bq + 8 * bq + p // 16] = 2.0
    Sg = np.zeros((128, 16 * NK), dtype=np.float16)
    for r in range(2):
        for t in range(NT):
            s = 8 * r + t
            for p in range(128):
                Sg[p, NK * s + 8 * t + p // 16] = -_sigma(r, t)
    I128 = np.eye(128, dtype=np.float16)
    Icol = np.zeros((128, NK), dtype=np.float16)
    for p in range(128):
        Icol[p, p % NK] = 1.0
    return S32, Sg, I128, Icol


def build_program():
    _patch_drain_wait_limit()
    nc = bass.Bass(
        "TRN2", target_bir_lowering=False, debug=False, num_devices=N_CORES
    )
    xT_d = nc.dram_tensor("xT", [F, JL], _FP16, kind="ExternalInput").ap()
    T_d = nc.dram_tensor("T", [F, KD], _FP16, kind="ExternalInput").ap()
    S32_d = nc.dram_tensor("S32", [128, 128], _FP16, kind="ExternalInput").ap()
    Sg_d = nc.dram_tensor("Sg", [128, 16 * NK], _FP16, kind="ExternalInput").ap()
    I128_d = nc.dram_tensor("I128", [128, 128], _FP16, kind="ExternalInput").ap()
    Icol_d = nc.dram_tensor("Icol", [128, NK], _FP16, kind="ExternalInput").ap()
    orow_d = nc.dram_tensor("orow", [128, NP], _FP32, kind="ExternalOutput").ap()
    ocol_d = nc.dram_tensor("ocol", [NK, JL], _FP32, kind="ExternalOutput").ap()
    e1_d = nc.dram_tensor("e1", [NK, NP], _FP32, kind="ExternalOutput").ap()

    AF = mybir.ActivationFunctionType
    AO = mybir.AluOpType

    with tile.TileContext(nc) as tc:
        with (
            tc.tile_pool(name="tw", bufs=NF) as t_pool,
            tc.tile_pool(name="xw", bufs=NF) as x_pool,
            tc.tile_pool(name="mt", bufs=NT) as mt_pool,
            tc.tile_pool(name="mc", bufs=1) as mc_pool,
            tc.tile_pool(name="ssb", bufs=1) as s_pool,
            tc.tile_pool(name="df", bufs=16 * (COL_LAG + 2)) as d_pool,
            tc.tile_pool(name="ep", bufs=COL_LAG + 2) as e_pool,
            tc.tile_pool(name="op", bufs=1) as o_pool,
            tc.tile_pool(name="pmm", bufs=2, space="PSUM") as psum_mm,
            tc.tile_pool(name="pgg", bufs=1, space="PSUM") as psum_g,
            tc.tile_pool(name="pl1", bufs=EXP_LAG + 2, space="PSUM") as psum_l1,
            tc.tile_pool(name="pcl", bufs=1, space="PSUM") as psum_col,
        ):
            # ---- loads (already fp16 host-side) ----
            T_t = []
            for f in range(NF):
                tt = t_pool.tile([128, KD], _FP16, tag="tw")
                nc.sync.dma_start(out=tt, in_=T_d[f * 128 : (f + 1) * 128, :])
                T_t.append(tt)
            x_t = []
            for f in range(NF):
                xt = x_pool.tile([128, JL], _FP16, tag="xw")
                nc.sync.dma_start(out=xt, in_=xT_d[f * 128 : (f + 1) * 128, :])
                x_t.append(xt)
            S32 = s_pool.tile([128, 128], _FP16, tag="s32")
            nc.sync.dma_start(out=S32, in_=S32_d)
            Sg = s_pool.tile([128, 16 * NK], _FP16, tag="sg")
            nc.sync.dma_start(out=Sg, in_=Sg_d)
            I128 = s_pool.tile([128, 128], _FP16, tag="i128")
            nc.sync.dma_start(out=I128, in_=I128_d)
            Icol = s_pool.tile([128, NK], _FP16, tag="icol")
            nc.sync.dma_start(out=Icol, in_=Icol_d)
            Zw = s_pool.tile([1, NK], _FP16, tag="zw")
            nc.vector.memset(Zw, 0.0)

            # init pcol: zero-weights matmul clears has_written + writes 0
            pcol = psum_col.tile([NK, JL], _FP32, tag="pcl")
            nc.tensor.matmul(
                pcol, lhsT=Zw, rhs=x_t[0][0:1, :], start=True, stop=False,
                skip_group_check=True,
            )

            # ---- phase 1: M^T tiles [128 kd, JL j] fp16 + fp32 own-row
            # columns (subtract scalars / relu biases) ----
            mt = []
            mc = []
            for t in range(NT):
                pm = psum_mm.tile([128, JL], _FP32, tag="pmm")
                for f in range(NF):
                    nc.tensor.matmul(
                        pm,
                        lhsT=T_t[f][:, t * 128 : (t + 1) * 128],
                        rhs=x_t[f],
                        start=(f == 0),
                        stop=(f == NF - 1),
                    )
                m = mt_pool.tile([128, JL], _FP16, tag="mt")
                cn = mc_pool.tile([128, NI], _FP32, tag=f"mc{t}")
                if t % 2 == 0:
                    nc.vector.tensor_copy(m, pm)
                    nc.scalar.copy(cn, pm[:, 0:NI])
                else:
                    nc.scalar.copy(m, pm)
                    nc.vector.tensor_copy(cn, pm[:, 0:NI])
                mt.append(m)
                mc.append(cn)

            # ---- phase 1.5: Gn2[64r + k, j] = -sigma(r,t(k)) * G[k, j] ----
            pg2 = psum_g.tile([128, JL], _FP32, tag="pgg")
            for r in range(2):
                for t in range(NT):
                    s = 8 * r + t
                    nc.tensor.matmul(
                        pg2[64 * r : 64 * r + 64, :],
                        lhsT=Sg[:, NK * s : NK * (s + 1)],
                        rhs=mt[t],
                        start=(t == 0),
                        stop=(t == NT - 1),
                        skip_group_check=True,
                    )
            Gn2 = s_pool.tile([128, JL], _FP16, tag="gn2")
            nc.vector.tensor_copy(Gn2, pg2)
            # exp bias: Gb2[p, m] = Gn2[p, 2m + p//64]
            Gb2 = s_pool.tile([128, NP], _FP32, tag="gb2")
            nc.vector.tensor_copy(Gb2[0:64, :], pg2[0:64, 0:NI:2])
            nc.vector.tensor_copy(Gb2[64:128, :], pg2[64:128, 1:NI:2])

            O_row = o_pool.tile([128, NP], _FP32, tag="orow")
            O_col = o_pool.tile([NK, JL], _FP32, tag="ocol")

            # ---- d1 pass: within-pair {2m, 2m+1} -> e1 (true |z|, weight-2
            # S patterns compensated by exp scale=-0.5) ----
            ds_all = d_pool.tile([128, NT * NP], _FP16, tag="d1s", bufs=1)
            for t in range(NT):
                nc.vector.tensor_tensor(
                    ds_all[:, NP * t : NP * (t + 1)],
                    mt[t][:, 0:NI:2],
                    mt[t][:, 1:NI:2],
                    op=AO.subtract,
                )
            da_all = d_pool.tile([128, NT * NP], _FP16, tag="d1a", bufs=1)
            nc.scalar.activation(da_all, ds_all, AF.Abs)
            pd1 = psum_l1.tile([NK, NP], _FP32, tag="pl1")
            for h in range(2):
                for bq in range(4):
                    t = 4 * h + bq
                    nc.tensor.matmul(
                        pd1[32 * h : 32 * h + 32, :],
                        lhsT=S32[:, 32 * bq : 32 * bq + 32],
                        rhs=da_all[:, NP * t : NP * (t + 1)],
                        start=(bq == 0),
                        stop=(bq == 3),
                        skip_group_check=True,
                        tile_position=(0, 32 * h),
                    )
            E1 = e_pool.tile([NK, NP], _FP32, tag="e1")
            nc.scalar.activation(E1, pd1, AF.Exp, scale=-0.5)
            nc.sync.dma_start(out=e1_d, in_=E1)

            # ---- phase 2: 32 pairs ----
            l1_tiles = [None] * NP
            e_tiles = [None] * NP

            def emit_exp(m):
                E = e_pool.tile([128, W], _FP16, tag="ep")
                nc.scalar.activation(
                    E, l1_tiles[m], AF.Exp, scale=-1.0,
                    bias=Gb2[:, m : m + 1],
                    accum_out=O_row[:, m : m + 1],
                )
                e_tiles[m] = E

            def emit_colacc(m):
                w0 = 2 * m + 2
                nc.tensor.matmul(
                    pcol[:, w0 : w0 + W - 2],
                    lhsT=Icol,
                    rhs=e_tiles[m][:, 0 : W - 2],
                    start=False,
                    stop=(m == NP - 1),
                    skip_group_check=True,
                )

            for m in range(NP):
                w0 = 2 * m + 2
                w1 = w0 + W
                diffs = [[None] * NT, [None] * NT]
                for r in range(2):
                    rc = 2 * m + r
                    for t in range(NT):
                        ab = d_pool.tile([128, W], _FP16, tag="df")
                        col = mc[t][:, rc : rc + 1]
                        if (r, t) in ACT_SLOTS:
                            nc.scalar.activation(
                                ab, mt[t][:, w0:w1], AF.Relu,
                                bias=col, scale=-1.0,
                            )
                        else:
                            nc.vector.tensor_scalar(
                                ab, mt[t][:, w0:w1], col, 0.0,
                                op0=AO.subtract, op1=AO.max,
                            )
                        diffs[r][t] = ab
                l1 = psum_l1.tile([128, W], _FP32, tag="pl1")
                for bq in range(4):
                    for r in range(2):
                        for h in range(2):
                            t = 4 * h + bq
                            nc.tensor.matmul(
                                l1[64 * r + 32 * h : 64 * r + 32 * h + 32, :],
                                lhsT=S32[:, 32 * bq : 32 * bq + 32],
                                rhs=diffs[r][t],
                                start=(bq == 0),
                                stop=False,
                                skip_group_check=True,
                                tile_position=(0, 64 * r + 32 * h),
                            )
                nc.tensor.matmul(
                    l1, lhsT=I128, rhs=Gn2[:, w0:w1],
                    start=False, stop=True, skip_group_check=True,
                )
                l1_tiles[m] = l1
                if m >= EXP_LAG:
                    emit_exp(m - EXP_LAG)
                if m >= COL_LAG:
                    emit_colacc(m - COL_LAG)
            for m in range(NP - EXP_LAG, NP):
                emit_exp(m)
            for m in range(NP - COL_LAG, NP):
                emit_colacc(m)

            nc.vector.tensor_copy(O_col, pcol)
            nc.sync.dma_start(out=orow_d, in_=O_row)
            nc.sync.dma_start(out=ocol_d, in_=O_col)
    return nc


_CACHED = {}


def _get_program():
    if "nc" not in _CACHED:
        _CACHED["nc"] = build_program()
        _CACHED["S"] = build_s_matrices()
    return _CACHED["nc"], _CACHED["S"]


def make_in_maps(x: np.ndarray, T: np.ndarray, S32, Sg, I128, Icol):
    xT = np.ascontiguousarray(x.T.astype(np.float16))
    T16 = np.ascontiguousarray(T.astype(np.float16))
    in_maps = []
    for c in range(N_CORES):
        xTc = np.ascontiguousarray(np.roll(xT, -NI * c, axis=1)[:, :JL])
        in_maps.append(
            {"xT": xTc, "T": T16, "S32": S32, "Sg": Sg, "I128": I128,
             "Icol": Icol}
        )
    return in_maps


def assemble(results) -> np.ndarray:
    out = np.zeros((B, NK), dtype=np.float64)
    for c in range(N_CORES):
        R = results[c]["orow"]  # [128, NP]: p<64 -> row 2m k=p; p>=64 -> 2m+1
        E1 = results[c]["e1"]  # [NK, NP]
        C = results[c]["ocol"]  # [NK, JL] local col sums
        base = NI * c
        out[base + 0 : base + NI : 2, :] += R[:NK, :].T + E1.T
        out[base + 1 : base + NI : 2, :] += R[NK:, :].T + E1.T
        Cfull = np.zeros((B, NK), dtype=np.float64)
        Cfull[:JL] = C.T
        Cfull[0:2] = 0.0  # never written by col-accumulate
        out += np.roll(Cfull, base, axis=0)
    return out.astype(np.float32)


def run(x: np.ndarray, T: np.ndarray, trace: bool = False):
    nc, S = _get_program()
    in_maps = make_in_maps(x, T, *S)
    res = bass_utils.run_bass_kernel_spmd(
        nc, in_maps, core_ids=list(range(N_CORES)), trace=trace
    )
    return assemble(res.results), res


def kernel(x: np.ndarray, T: np.ndarray) -> np.ndarray:
    out, _ = run(x, T)
    return out
